# revision 11
# baseline (speedup 1.0000x reference)
"""Causal self-attention (GQA + RoPE) Trainium2 kernel, v2.

Full-input contract: kernel(**inputs) takes the unsharded tensors and returns
the full [B, T, C] output. Internally shards over 8 NeuronCores as
(batch b in {0,1}) x (kv-head group g in {0..3}); each core computes the
attention output of its 4 query heads (one kv head) for its batch and the
partial out-projection against its 512 rows of Wo. The host sums the 4 group
partials per batch.

v2 structure (single TileContext scope, no mid-kernel barrier):
  - PSUM is one pool with shared tags across phases: "q" (4 bufs: Q-proj,
    score tiles, out-proj), "kv" (2: K/V-proj, attn@V accum), "sp" (2:
    V-transpose scratch, softmax denominators).
  - RoPE reads the projection PSUM directly on DVE, using cross-partition
    operand slices for the rotate-half term (PSUM in0 may have a different
    base partition than the SBUF in1 - verified on HW).
  - Causal F-restriction on score, exp, denominator and attn@V ops.
  - Softmax denominator via all-ones stationary matmul (PSUM-accumulated
    across k tiles), normalization as reciprocal*mul on DVE.
  - Phase B is software-pipelined: scores of tile i+1 are emitted between
    scores(i) and denom/attnV(i) so the mask->exp chain latency is hidden;
    the first two score groups of the next q-block are peeled ahead of the
    out-projection to cover the normalization latency.
  - Out-projection runs in bf16 (stationary ot tile, moving Wo), fp32 PSUM.
  - Weights/x are loaded with batched 3D-AP DMAs spread over the sync,
    gpsimd, scalar and tensor queues; y is stored as 1 MB row blocks.
"""

import sys

for _p in ("/opt/trn_rl_repo", "/root/.axon_site/_ro/trn_rl_repo"):
    if _p not in sys.path:
        sys.path.append(_p)

import numpy as np
from contextlib import ExitStack

import concourse.bass as bass
import concourse.bacc as bacc
import concourse.tile as tile
import concourse.mybir as mybir
from concourse.bass_utils import run_bass_kernel_spmd

F32 = mybir.dt.float32
F32R = mybir.dt.float32r
BF16 = mybir.dt.bfloat16
U16 = mybir.dt.uint16

B, T, C = 2, 2048, 2048
N_HEADS, N_KV_HEADS, HD = 16, 4, 128
G = N_HEADS // N_KV_HEADS  # heads per group = 4
GW = G * HD  # 512, per-group Q width / Wo row count
N_CORES = 8
TC = 512  # q-block width
NTC = T // TC  # 4
NCC = C // 128  # 16 contraction chunks
MASK_NEG = -1.0e30

_prog_cache = {}


def _build_program():
    nc = bacc.Bacc(
        "TRN2",
        target_bir_lowering=False,
        debug=False,
        enable_asserts=False,
        num_devices=N_CORES,
    )

    xT = nc.dram_tensor("xT", [C, T], F32, kind="ExternalInput").ap()
    wq = nc.dram_tensor("wq", [C, GW], F32, kind="ExternalInput").ap()
    wk = nc.dram_tensor("wk", [C, HD], F32, kind="ExternalInput").ap()
    wv = nc.dram_tensor("wv", [C, HD], F32, kind="ExternalInput").ap()
    wo = nc.dram_tensor("wo", [GW, C], U16, kind="ExternalInput").ap()  # bf16 bits
    cos = nc.dram_tensor("cos", [HD, T], F32, kind="ExternalInput").ap()
    sin = nc.dram_tensor("sin", [HD, T], F32, kind="ExternalInput").ap()
    masks = nc.dram_tensor("masks", [128, 128], F32, kind="ExternalInput").ap()
    ident = nc.dram_tensor("ident", [128, 128], F32, kind="ExternalInput").ap()
    onesfull = nc.dram_tensor("onesfull", [128, 128], F32, kind="ExternalInput").ap()
    y = nc.dram_tensor("y", [T, C], F32, kind="ExternalOutput").ap()

    with tile.TileContext(nc) as tc, ExitStack() as ctx:
        sb = ctx.enter_context(tc.tile_pool(name="sb", bufs=1))
        xin = ctx.enter_context(tc.tile_pool(name="xin", bufs=3))
        rp = ctx.enter_context(tc.tile_pool(name="rp", bufs=2))
        ptp = ctx.enter_context(tc.tile_pool(name="ptp", bufs=6))
        nrm = ctx.enter_context(tc.tile_pool(name="nrm", bufs=2))
        otp = ctx.enter_context(tc.tile_pool(name="otp", bufs=2))
        ysb = ctx.enter_context(tc.tile_pool(name="ysb", bufs=2))
        ps = ctx.enter_context(tc.tile_pool(name="ps", bufs=1, space="PSUM"))

        # persistent activations
        qt_sb = sb.tile([128, G, T], F32R)  # [d, head, t]
        kt_sb = sb.tile([128, T], F32R)
        v_sb = sb.tile([128, T // 128, HD], F32R)  # [t-part, kt, d]

        # weights / tables
        wq_sb = sb.tile([128, NCC, GW], F32R)
        wk_sb = sb.tile([128, NCC, HD], F32R)
        wv_sb = sb.tile([128, NCC, HD], F32R)
        cos_sb = sb.tile([HD, T], F32)
        sin_sb = sb.tile([HD, T], F32)
        ident_sb = sb.tile([128, 128], F32)
        mask_sb = sb.tile([128, 128], F32)
        ones_sb = sb.tile([128, 128], F32R)
        wo_sb = sb.tile([128, G, C], BF16)

        # ---- weight prefetch. Small 2D DMAs (3D gather APs choke the DMA
        # issue queue). wq0 + early wk/wv chunks on sync so the first
        # matmuls start ASAP; the rest of wq as per-ci singles on scalar.
        wk3 = wk.rearrange("(c p) j -> p c j", p=128)
        wv3 = wv.rearrange("(c p) j -> p c j", p=128)
        wq3 = wq.rearrange("(c p) j -> p c j", p=128)

        nc.sync.dma_start(wq_sb[:, 0:1, :], wq3[:, 0:1, :].bitcast(F32R))
        nc.sync.dma_start(ident_sb[:], ident)
        for ci in range(1, NCC):
            nc.scalar.dma_start(
                wq_sb[:, ci : ci + 1, :], wq3[:, ci : ci + 1, :].bitcast(F32R)
            )
        nc.scalar.dma_start(cos_sb[:], cos)
        nc.scalar.dma_start(sin_sb[:], sin)

        # ---------------- phase A: projections + rope ----------------
        for tci in range(NTC):
            ts = slice(tci * TC, (tci + 1) * TC)
            qt_ps = [
                ps.tile([128, TC], F32, tag="q", bufs=4, name=f"qtps{tci}_{j}")
                for j in range(G)
            ]
            kt_ps = ps.tile([128, TC], F32, tag="kv", bufs=2, name=f"ktps{tci}")
            vt_ps = ps.tile([128, TC], F32, tag="kv", bufs=2, name=f"vtps{tci}")
            for cp in range(NCC // 2):
                x_t = xin.tile([128, 2, TC], F32R, tag="x", name=f"x{tci}_{cp}")
                r0 = (2 * cp) * 128
                nc.sync.dma_start(
                    x_t[:],
                    xT[r0 : r0 + 256, ts]
                    .rearrange("(c p) t -> p c t", c=2)
                    .bitcast(F32R),
                )
                if tci == 0:
                    # stagger K/V weight chunks between the x loads on sync
                    if cp == 0:
                        nc.sync.dma_start(
                            wk_sb[:, 0:4, :], wk3[:, 0:4, :].bitcast(F32R)
                        )
                        nc.sync.dma_start(
                            wv_sb[:, 0:4, :], wv3[:, 0:4, :].bitcast(F32R)
                        )
                    elif cp == 1:
                        nc.sync.dma_start(
                            wk_sb[:, 4:16, :], wk3[:, 4:16, :].bitcast(F32R)
                        )
                        nc.sync.dma_start(
                            wv_sb[:, 4:16, :], wv3[:, 4:16, :].bitcast(F32R)
                        )
                if tci == 1 and cp == 0:
                    nc.scalar.dma_start(mask_sb[:], masks)
                    nc.scalar.dma_start(ones_sb[:], onesfull.bitcast(F32R))
                if tci == 1 and 1 <= cp <= 4:
                    h = cp - 1
                    nc.scalar.dma_start(
                        wo_sb[:, h, :], wo[h * 128 : (h + 1) * 128, :].bitcast(BF16)
                    )
                for sub in range(2):
                    ci = 2 * cp + sub
                    xs = x_t[:, sub, :]
                    st, sp = (ci == 0), (ci == NCC - 1)
                    for j in range(G):
                        nc.tensor.matmul(
                            qt_ps[j][:],
                            wq_sb[:, ci, j * HD : (j + 1) * HD],
                            xs,
                            start=st,
                            stop=sp,
                        )
                    nc.tensor.matmul(kt_ps[:], wk_sb[:, ci, :], xs, start=st, stop=sp)
                    nc.tensor.matmul(vt_ps[:], wv_sb[:, ci, :], xs, start=st, stop=sp)

            # rope: qt = q*cos + swap_half(q)*sin_signed. Stage q out of
            # PSUM with one scalar copy (frees the bank for the next tci in
            # ~0.6us instead of after three DVE reads), build the half-swap
            # with cross-partition scalar copies, then two muls + add on DVE.
            def rope(src_ps, dst, idx):
                q_raw = rp.tile([128, TC], F32, tag="qraw", name=f"qraw{idx}")
                nc.scalar.copy(q_raw[:], src_ps[:])
                qsw = rp.tile([128, TC], F32, tag="qsw", name=f"qsw{idx}")
                nc.scalar.copy(qsw[0:64, :], q_raw[64:128, :])
                nc.scalar.copy(qsw[64:128, :], q_raw[0:64, :])
                t1 = rp.tile([128, TC], F32, tag="t1", name=f"t1_{idx}")
                nc.vector.tensor_mul(t1[:], q_raw[:], cos_sb[:, ts])
                t2 = rp.tile([128, TC], F32, tag="t2", name=f"t2_{idx}")
                nc.vector.tensor_mul(t2[:], qsw[:], sin_sb[:, ts])
                nc.vector.tensor_add(dst, t1[:], t2[:])

            for j in range(G):
                rope(qt_ps[j], qt_sb[:, j, ts], f"{tci}_{j}")
            rope(kt_ps, kt_sb[:, ts], f"k{tci}")

            # V: [d, t] psum -> sbuf, then PE-transpose to [t, d]
            vt_f = rp.tile([128, TC], F32, tag="vtf", name=f"vtf{tci}")
            nc.scalar.copy(vt_f[:], vt_ps[:])
            for s in range(TC // 128):
                kt_i = tci * (TC // 128) + s
                tp_t = ps.tile([128, TC], F32, tag="sp", bufs=2, name=f"tp{kt_i}")
                nc.tensor.transpose(
                    tp_t[:, 0:128], vt_f[:, s * 128 : (s + 1) * 128], ident_sb[:]
                )
                nc.scalar.copy(v_sb[:, kt_i, :], tp_t[:, 0:128])

        # -------- phase B: attention + out-projection per q-block --------
        # pending[(hg,)] closures emitted with a software-pipeline distance
        # of 2 between the score group S(i) and its denom/attnV group D(i).
        def make_block(qb, hg):
            # diag tiles first so PSUM start flag covers full columns
            kts = list(range(4 * qb, 4 * qb + 4)) + list(range(0, 4 * qb))
            n = len(kts)
            sb_ps = [
                ps.tile([128, TC], F32, tag="sp", bufs=2, name=f"sps{qb}_{hg}_{i}")
                for i in range(2)
            ]
            ot_ps = [
                ps.tile([128, TC], F32, tag="kv", bufs=2, name=f"otps{qb}_{hg}_{i}")
                for i in range(2)
            ]
            pts = {}

            def S(i):
                kt = kts[i]
                dj = kt - 4 * qb
                f0 = max(dj, 0) * 128
                cur = []
                for hh in range(2):
                    h = 2 * hg + hh
                    s_t = ps.tile(
                        [128, TC], F32, tag="q", bufs=4, name=f"st{qb}_{kt}_{h}"
                    )
                    nc.tensor.matmul(
                        s_t[:, f0:TC],
                        kt_sb[:, kt * 128 : (kt + 1) * 128],
                        qt_sb[:, h, qb * TC + f0 : (qb + 1) * TC],
                        start=True,
                        stop=True,
                    )
                    if dj >= 0:
                        nc.vector.tensor_add(
                            s_t[:, f0 : f0 + 128], s_t[:, f0 : f0 + 128], mask_sb[:]
                        )
                    pt = ptp.tile([128, TC], F32R, tag="pt", name=f"pt{qb}_{kt}_{h}")
                    nc.scalar.activation(
                        pt[:, f0:TC], s_t[:, f0:TC], mybir.ActivationFunctionType.Exp
                    )
                    cur.append(pt)
                pts[i] = (cur, f0)

            def D(i):
                kt = kts[i]
                cur, f0 = pts.pop(i)
                st_, sp_ = (i == 0), (i == n - 1)
                for hh in range(2):
                    nc.tensor.matmul(
                        sb_ps[hh][:, f0:TC],
                        ones_sb[:],
                        cur[hh][:, f0:TC],
                        start=st_,
                        stop=sp_,
                    )
                    nc.tensor.matmul(
                        ot_ps[hh][:, f0:TC],
                        v_sb[:, kt, :],
                        cur[hh][:, f0:TC],
                        start=st_,
                        stop=sp_,
                    )

            return S, D, n, sb_ps, ot_ps

        blocks = {}

        def emit_block(qb, hg, peeled, dve_filler=()):
            S, D, n, sb_ps, ot_ps = blocks[(qb, hg)]
            filler = list(dve_filler)

            def fill():
                if filler:
                    filler.pop(0)()

            for i in range(peeled, n):
                S(i)
                if i >= 2:
                    D(i - 2)
                if i >= 1:
                    fill()
            D(n - 2)
            fill()
            D(n - 1)
            while filler:
                filler.pop(0)()
            return sb_ps, ot_ps

        for qb in range(NTC):
            if (qb, 0) not in blocks:
                blocks[(qb, 0)] = make_block(qb, 0)
            blocks[(qb, 1)] = make_block(qb, 1)
            peeled = 2 if qb > 0 else 0

            # per-head normalized attention outputs (separate tiles so the
            # out-projection's h-loop tracks each head's norm individually)
            ot_t = [
                otp.tile([128, TC], BF16, tag=f"ot{h}", bufs=2, name=f"ot{qb}_{h}")
                for h in range(G)
            ]

            def norm_ops(hg, sbp, otps):
                rfs = {}

                def mk(hh):
                    h = 2 * hg + hh

                    def op_r():
                        r_f = nrm.tile([128, TC], F32, tag="rf", name=f"rf{qb}_{h}")
                        nc.vector.reciprocal_approx_fast(r_f[:], sbp[hh][:])
                        rfs[h] = r_f

                    def op_m():
                        nc.vector.tensor_mul(ot_t[h][:], otps[hh][:], rfs[h][:])

                    return [op_r, op_m]

                return mk(0) + mk(1)

            sb0, ot0 = emit_block(qb, 0, peeled)
            # hg0's normalization rides along inside hg1's pipeline
            sb1, ot1 = emit_block(qb, 1, 0, dve_filler=norm_ops(0, sb0, ot0))

            # peel the first two score groups of the next q-block ahead of
            # the normalization + out-projection to keep the PE fed
            if qb + 1 < NTC:
                blocks[(qb + 1, 0)] = make_block(qb + 1, 0)
                Sn, _, _, _, _ = blocks[(qb + 1, 0)]
                Sn(0)
                Sn(1)
            for op in norm_ops(1, sb1, ot1):
                op()

            # out-projection for this q-block, stored as 1MB row blocks
            for tl in range(TC // 128):
                tsub = qb * (TC // 128) + tl
                y_sb = ysb.tile([128, C], F32, tag="y", name=f"ysb{tsub}")
                last = tsub == T // 128 - 1
                for cc in range(C // TC):
                    y_ps = ps.tile(
                        [128, TC], F32, tag="q", bufs=4, name=f"yps{tsub}_{cc}"
                    )
                    for h in range(G):
                        nc.tensor.matmul(
                            y_ps[:],
                            ot_t[h][:, tl * 128 : (tl + 1) * 128],
                            wo_sb[:, h, cc * TC : (cc + 1) * TC],
                            start=(h == 0),
                            stop=(h == G - 1),
                        )
                    nc.vector.tensor_copy(y_sb[:, cc * TC : (cc + 1) * TC], y_ps[:])
                    if last:
                        yq = (nc.sync, nc.scalar, nc.sync, nc.scalar)[cc]
                        yq.dma_start(
                            y[tsub * 128 : (tsub + 1) * 128, cc * TC : (cc + 1) * TC],
                            y_sb[:, cc * TC : (cc + 1) * TC],
                        )
                if not last:
                    yq = nc.sync if tsub % 2 == 0 else nc.scalar
                    yq.dma_start(y[tsub * 128 : (tsub + 1) * 128, :], y_sb[:])

    nc.compile()
    return nc


def _rope_tables():
    theta = 1.0 / (10000.0 ** (np.arange(0, HD, 2, dtype=np.float32) / HD))
    freqs = np.arange(T, dtype=np.float32)[:, None] * theta[None, :]  # [T, 64]
    cos = np.concatenate([np.cos(freqs), np.cos(freqs)], axis=-1)  # [T, 128]
    sin = np.concatenate([np.sin(freqs), np.sin(freqs)], axis=-1)
    cosT = np.ascontiguousarray(cos.T).astype(np.float32)  # [128, T]
    sinT = np.ascontiguousarray(sin.T).astype(np.float32)
    sign = np.where(np.arange(HD) < HD // 2, np.float32(-1.0), np.float32(1.0))[:, None]
    sinT_signed = (sinT * sign).astype(np.float32)
    return cosT, sinT_signed


def _masks():
    p = np.arange(128)[:, None]
    f = np.arange(128)[None, :]
    return np.where(p <= f, 0.0, MASK_NEG).astype(np.float32)


def _to_bf16_bits(a):
    # round-to-nearest-even fp32 -> bf16, returned as uint16 bit pattern
    u = np.asarray(a, dtype=np.float32).view(np.uint32).astype(np.uint64)
    rounded = (u + 0x7FFF + ((u >> 16) & 1)) >> 16
    return rounded.astype(np.uint16)


def make_in_maps(x, Wq, Wk, Wv, Wo):
    x = np.asarray(x, dtype=np.float32)
    Wq = np.asarray(Wq, dtype=np.float32)
    Wk = np.asarray(Wk, dtype=np.float32)
    Wv = np.asarray(Wv, dtype=np.float32)
    Wo = np.asarray(Wo, dtype=np.float32)

    cosT, sinT = _rope_tables()
    masks = _masks()
    qscale = np.float32(1.0 / np.sqrt(HD))
    ident = np.eye(128, dtype=np.float32)
    onesfull = np.ones((128, 128), dtype=np.float32)

    in_maps = []
    for c in range(N_CORES):
        b, g = divmod(c, N_KV_HEADS)
        in_maps.append(
            {
                "xT": np.ascontiguousarray(x[b].T),
                "wq": np.ascontiguousarray(Wq[:, g * GW : (g + 1) * GW]) * qscale,
                "wk": np.ascontiguousarray(Wk[:, g * HD : (g + 1) * HD]),
                "wv": np.ascontiguousarray(Wv[:, g * HD : (g + 1) * HD]),
                "wo": _to_bf16_bits(Wo[g * GW : (g + 1) * GW, :]),
                "cos": cosT,
                "sin": sinT,
                "masks": masks,
                "ident": ident,
                "onesfull": onesfull,
            }
        )
    return in_maps


def kernel(x, Wq, Wk, Wv, Wo):
    if "nc" not in _prog_cache:
        _prog_cache["nc"] = _build_program()
    nc = _prog_cache["nc"]

    in_maps = make_in_maps(x, Wq, Wk, Wv, Wo)
    res = run_bass_kernel_spmd(nc, in_maps, list(range(N_CORES)))
    _prog_cache["last_results"] = res

    out = np.zeros((B, T, C), dtype=np.float32)
    for c in range(N_CORES):
        b = c // N_KV_HEADS
        out[b] += res.results[c]["y"]
    return out
